# revision 20
# baseline (speedup 1.0000x reference)
"""Additive attention (Bahdanau-style) Trainium2 kernel, 8 NeuronCores.

reference:
    h = tanh(key @ Wk^T + query @ Wq^T + bias)          # [B, L, H]
    score = h @ w_score + b_score                        # [B, L]
    attn = softmax(score, -1)                            # [B, L]
    context = einsum('bl,blh->bh', attn, value)[:,None]  # [B, 1, H]
    returns (context, attn)

Sharding: data-parallel over batch B=16 -> 2 batches per core, no collectives.
b_score cancels inside softmax and is dropped.

Per-core layout (T = 2*2048 = 4096 tokens, H = 1024):
  - k/q are DMA-transposed on load (xbar, bf16) into XT [h=128p, hc, tok]
  - projections run on PE as out[tok,o] = sum_h XT[h,tok]^T @ WcatT[h,o]
    with WcatT = concat(Wk.T, Wq.T) [2048, 1024] so both projections are one
    16-chunk accumulation
  - bias add (DVE) + tanh (ACT) + fused mul+reduce against w_score (DVE)
    produce score^T [tok=128p, 16] per batch
  - exp with per-partition accumulation (ACT), Z via ones-matmul, reciprocal
    broadcast, then the value matvec uses pre-normalized exp weights so the
    context comes out of PSUM already normalized
"""
import sys

sys.path.insert(0, "/opt/trn_rl_repo")

import numpy as np
import ml_dtypes

import concourse.bass as bass
import concourse.mybir as mybir
import concourse.tile as tile
from concourse import bacc
from concourse import bass_utils

BF16 = ml_dtypes.bfloat16
F32 = mybir.dt.float32
BF = mybir.dt.bfloat16

B, L, H = 16, 2048, 1024
NCORES = 8
BPC = B // NCORES          # batches per core = 2
T = BPC * L                # tokens per core = 4096
P = 128
HC = (2 * H) // P          # 16 contraction chunks (key 0..7, query 8..15)
OC = H // 512              # 2 free-dim halves of the output projection
NT = L // P                # 16 token tiles per batch
CHUNK = 256                # token chunk for transposed loads
NJ = CHUNK // P            # 4 token tiles per chunk


def build_nc():
    nc = bacc.Bacc(None, target_bir_lowering=False, debug=False)

    k_h = nc.dram_tensor("k", [T, H], BF, kind="ExternalInput")
    q_h = nc.dram_tensor("q", [T, H], BF, kind="ExternalInput")
    v_h = nc.dram_tensor("v", [T, H], BF, kind="ExternalInput")
    wcat_h = nc.dram_tensor("wcat", [2 * H, H], BF, kind="ExternalInput")
    biasw_h = nc.dram_tensor("biasw", [P, H], F32, kind="ExternalInput")
    wsc_h = nc.dram_tensor("wsc", [P, H], F32, kind="ExternalInput")
    nws_h = nc.dram_tensor("nws", [P, 1], F32, kind="ExternalInput")
    ident_h = nc.dram_tensor("ident", [P, P], F32, kind="ExternalInput")
    octx_h = nc.dram_tensor("octx", [BPC, H], F32, kind="ExternalOutput")
    oattn_h = nc.dram_tensor("oattn", [BPC, L], F32, kind="ExternalOutput")

    # per-batch chunk plan (token offsets within the batch): a small first
    # chunk on batch 0 so the first transposes land quickly
    def chunk_plan(b):
        if b == 0:
            return [(0, 256), (256, 256), (512, 512), (1024, 512), (1536, 512)]
        return [(off, 512) for off in range(0, L, 512)]

    with tile.TileContext(nc) as tc:
        with (
            tc.tile_pool(name="const", bufs=1) as const,
            tc.tile_pool(name="xt", bufs=3) as xtp,
            tc.tile_pool(name="vp", bufs=2) as vp,
            tc.tile_pool(name="work", bufs=4) as work,
            tc.tile_pool(name="up", bufs=2) as up,
            tc.tile_pool(name="small", bufs=4) as small,
            tc.tile_pool(name="psproj", bufs=4, space="PSUM") as psproj,
            tc.tile_pool(name="psctx", bufs=2, space="PSUM") as psctx,
            tc.tile_pool(name="pstiny", bufs=2, space="PSUM") as pstiny,
        ):
            # ---- HAM warm-up: dummy matmuls so the PE clock is ramping
            # toward 2.4 GHz while the first operands stream in.
            ones = const.tile([P, P], F32, tag="ones")
            nc.vector.memset(ones[:], 1.0)
            warm_ps = pstiny.tile([P, 1], F32, tag="tiny")
            for _ in range(32):
                nc.tensor.matmul(
                    warm_ps[:], ones[:], ones[:, 0:1], start=True, stop=True
                )

            # ---- transposed k/q loads, prefetchable ----
            xt_tiles = {}

            def issue_xt(b, ci):
                key = (b, ci)
                if key not in xt_tiles:
                    off, size = chunk_plan(b)[ci]
                    tt = b * L + off
                    xtk = xtp.tile([P, H // P, size], BF, tag="xtk")
                    nc.sync.dma_start_transpose(xtk[:], k_h[tt : tt + size, :])
                    xtq = xtp.tile([P, H // P, size], BF, tag="xtq")
                    nc.sync.dma_start_transpose(xtq[:], q_h[tt : tt + size, :])
                    xt_tiles[key] = (xtk, xtq)
                return xt_tiles[key]

            def issue_xt_flat(fi):
                n0 = len(chunk_plan(0))
                if fi < n0:
                    return issue_xt(0, fi)
                fi -= n0
                if fi < len(chunk_plan(1)):
                    return issue_xt(1, fi)
                return None

            # first chunk's transposes go out before everything else; weights
            # follow as per-chunk slices so MM hc=0 isn't gated on the full
            # 4MB weight load.
            issue_xt(0, 0)
            wcat = const.tile([P, HC, H], BF, tag="wcat")
            for hc in range(HC):
                nc.sync.dma_start(wcat[:, hc, :], wcat_h[hc * P : (hc + 1) * P, :])
            biasw = const.tile([P, H], F32, tag="biasw")
            nc.sync.dma_start(biasw[:], biasw_h[:, :])
            wsc = const.tile([P, H], F32, tag="wsc")
            nc.sync.dma_start(wsc[:], wsc_h[:, :])
            nws = const.tile([P, 1], F32, tag="nws")
            nc.sync.dma_start(nws[:], nws_h[:, :])
            ident = const.tile([P, P], F32, tag="ident")
            nc.sync.dma_start(ident[:], ident_h[:, :])

            flat_base = 0
            for b in range(BPC):
                t0 = b * L
                plan = chunk_plan(b)
                # value for this batch, natural layout [p, tile, h];
                # loaded chunk-by-chunk below so it never starves the
                # transpose stream
                v_sb = vp.tile([P, NT, H], BF, tag="v")

                score = up.tile([P, NT], F32, tag="score")
                sparts = up.tile([P, 2 * NT], F32, tag="sparts")
                u_f = up.tile([P, NT], F32, tag="uf")
                u_bf = up.tile([P, NT], BF, tag="ubf")
                usump = up.tile([P, len(plan)], F32, tag="usump")

                for ci, (off, size) in enumerate(plan):
                    xtk, xtq = issue_xt(b, ci)
                    for ahead in (1, 2):
                        issue_xt_flat(flat_base + ci + ahead)
                    nc.sync.dma_start(
                        v_sb[:, off // P : (off + size) // P, :],
                        v_h[t0 + off : t0 + off + size, :].rearrange(
                            "(c p) h -> p c h", p=P
                        ),
                    )

                    nj = size // P
                    c0 = off // P
                    for j in range(nj):
                        col = c0 + j
                        js = slice(j * P, (j + 1) * P)
                        for half in range(OC):
                            hs = slice(half * 512, (half + 1) * 512)
                            ps = psproj.tile([P, 512], F32, tag="proj")
                            for hc in range(HC):
                                src = xtk if hc < 8 else xtq
                                nc.tensor.matmul(
                                    ps[:],
                                    src[:, hc % 8, js],
                                    wcat[:, hc, hs],
                                    start=(hc == 0),
                                    stop=(hc == HC - 1),
                                )
                            biased = work.tile([P, 512], F32, tag="biased")
                            nc.vector.tensor_add(biased[:], ps[:], biasw[:, hs])
                            # tanh(x) = 2*sigmoid(2x) - 1 (ACT Tanh faults on
                            # this terminal; Sigmoid is accurate). score_true =
                            # 2*sum(w*sig) - sum(w); the *2 and -sum(w) fold
                            # into the Exp's scale/bias.
                            th = work.tile([P, 512], F32, tag="tanh")
                            nc.scalar.activation(
                                th[:],
                                biased[:],
                                mybir.ActivationFunctionType.Sigmoid,
                                scale=2.0,
                            )
                            nc.vector.tensor_mul(th[:], th[:], wsc[:, hs])
                            nc.vector.reduce_sum(
                                out=sparts[:, 2 * col + half : 2 * col + half + 1],
                                in_=th[:],
                                axis=mybir.AxisListType.X,
                            )
                    # incremental softmax for this chunk's columns:
                    # u = exp(2*s + nws), per-chunk partial sums in usump
                    nc.vector.reduce_sum(
                        out=score[:, c0 : c0 + nj].rearrange(
                            "p (c t) -> p c t", t=1
                        ),
                        in_=sparts[:, 2 * c0 : 2 * (c0 + nj)].rearrange(
                            "p (c t) -> p c t", t=2
                        ),
                        axis=mybir.AxisListType.X,
                    )
                    nc.scalar.activation(
                        u_f[:, c0 : c0 + nj],
                        score[:, c0 : c0 + nj],
                        mybir.ActivationFunctionType.Exp,
                        bias=nws[:, 0:1],
                        scale=2.0,
                        accum_out=usump[:, ci : ci + 1],
                    )
                    nc.vector.tensor_copy(
                        u_bf[:, c0 : c0 + nj], u_f[:, c0 : c0 + nj]
                    )
                flat_base += len(plan)

                # ---- batch tail: Z, 1/Z, outputs ----
                usum = up.tile([P, 1], F32, tag="usum")
                nc.vector.reduce_sum(
                    out=usum[:], in_=usump[:], axis=mybir.AxisListType.X
                )
                zb = pstiny.tile([P, 1], F32, tag="tiny")
                nc.tensor.matmul(zb[:], ones[:], usum[:], start=True, stop=True)
                rz = small.tile([P, 1], F32, tag="rz")
                nc.vector.reciprocal(rz[:], zb[:])

                # ---- context = (1/Z) * sum_t u[t] * v[t, :] ----
                for half in range(OC):
                    hs = slice(half * 512, (half + 1) * 512)
                    cps = psctx.tile([1, 512], F32, tag="ctx")
                    for t in range(NT):
                        nc.tensor.matmul(
                            cps[:],
                            u_bf[:, t : t + 1],
                            v_sb[:, t, hs],
                            start=(t == 0),
                            stop=(t == NT - 1),
                        )
                    ctx_sb = small.tile([1, 512], F32, tag="ctxsb")
                    nc.vector.tensor_scalar_mul(ctx_sb[:], cps[:], rz[0:1, 0:1])
                    nc.sync.dma_start(octx_h[b, hs], ctx_sb[:])

                # ---- attn output: transpose u_f -> [16, 128] rows, then
                # scale by 1/Z per output row (rz is partition-replicated) ----
                at = pstiny.tile([NT, P], F32, tag="tiny")
                nc.tensor.transpose(at[:], u_f[:], ident[:])
                attn_sb = small.tile([NT, P], F32, tag="attnsb")
                nc.vector.tensor_scalar_mul(attn_sb[:], at[:], rz[0:NT, 0:1])
                nc.sync.dma_start(
                    oattn_h[b, :].rearrange("(c p) -> c p", c=NT), attn_sb[:]
                )

    nc.compile()
    return nc


_NC_CACHE = None


def _get_nc():
    global _NC_CACHE
    if _NC_CACHE is None:
        _NC_CACHE = build_nc()
    return _NC_CACHE


def _prep_inputs(query, key_t, value, Wq, Wk, bias, w_score):
    query = np.asarray(query, dtype=np.float32)
    key_t = np.asarray(key_t, dtype=np.float32)
    value = np.asarray(value, dtype=np.float32)
    k_b = key_t.reshape(NCORES, T, H).astype(BF16)
    q_b = query.reshape(NCORES, T, H).astype(BF16)
    v_b = value.reshape(NCORES, T, H).astype(BF16)
    wcat = np.ascontiguousarray(
        np.concatenate(
            [np.asarray(Wk, np.float32).T, np.asarray(Wq, np.float32).T], axis=0
        )
    ).astype(BF16)
    biasw = np.ascontiguousarray(
        np.tile(np.asarray(bias, np.float32)[None, :], (P, 1))
    )
    wsc = np.ascontiguousarray(
        np.tile(np.asarray(w_score, np.float32)[None, :], (P, 1))
    )
    nws = np.full((P, 1), -np.asarray(w_score, np.float64).sum(), dtype=np.float32)
    ident = np.eye(P, dtype=np.float32)
    return [
        {
            "k": np.ascontiguousarray(k_b[i]),
            "q": np.ascontiguousarray(q_b[i]),
            "v": np.ascontiguousarray(v_b[i]),
            "wcat": wcat,
            "biasw": biasw,
            "wsc": wsc,
            "nws": nws,
            "ident": ident,
        }
        for i in range(NCORES)
    ]


def kernel(query, key_t, value, Wq, Wk, bias, w_score, b_score, _trace=False):
    in_maps = _prep_inputs(query, key_t, value, Wq, Wk, bias, w_score)
    nc = _get_nc()
    res = bass_utils.run_bass_kernel_spmd(
        nc, in_maps, core_ids=list(range(NCORES)), trace=_trace
    )
    ctx = np.stack([r["octx"] for r in res.results]).reshape(B, H)[:, None, :]
    attn = np.stack([r["oattn"] for r in res.results]).reshape(B, L)
    if _trace:
        kernel.last_results = res
    return ctx, attn


if __name__ == "__main__":
    rng = np.random.default_rng(0)
    ins = {
        "query": rng.standard_normal((B, L, H), dtype=np.float32),
        "key_t": rng.standard_normal((B, L, H), dtype=np.float32),
        "value": rng.standard_normal((B, L, H), dtype=np.float32),
        "Wq": rng.uniform(-1 / 32, 1 / 32, (H, H)).astype(np.float32),
        "Wk": rng.uniform(-1 / 32, 1 / 32, (H, H)).astype(np.float32),
        "bias": rng.uniform(-0.1, 0.1, (H,)).astype(np.float32),
        "w_score": rng.uniform(-1 / 32, 1 / 32, (H,)).astype(np.float32),
        "b_score": rng.uniform(-1 / 32, 1 / 32, (1,)).astype(np.float32),
    }
    ctx, attn = kernel(**ins)
    print("ctx", ctx.shape, "attn", attn.shape, attn.sum(-1)[:4])


# revision 21
# speedup vs baseline: 1.0088x; 1.0088x over previous
"""Additive attention (Bahdanau-style) Trainium2 kernel, 8 NeuronCores.

reference:
    h = tanh(key @ Wk^T + query @ Wq^T + bias)          # [B, L, H]
    score = h @ w_score + b_score                        # [B, L]
    attn = softmax(score, -1)                            # [B, L]
    context = einsum('bl,blh->bh', attn, value)[:,None]  # [B, 1, H]
    returns (context, attn)

Sharding: data-parallel over batch B=16 -> 2 batches per core, no collectives.
b_score cancels inside softmax and is dropped.

Per-core layout (T = 2*2048 = 4096 tokens, H = 1024):
  - k/q are DMA-transposed on load (xbar, bf16) into XT [h=128p, hc, tok]
  - projections run on PE as out[tok,o] = sum_h XT[h,tok]^T @ WcatT[h,o]
    with WcatT = concat(Wk.T, Wq.T) [2048, 1024] so both projections are one
    16-chunk accumulation
  - bias add (DVE) + tanh (ACT) + fused mul+reduce against w_score (DVE)
    produce score^T [tok=128p, 16] per batch
  - incremental softmax: per-chunk exp with per-partition accumulation (ACT),
    Z via ones-matmul partition broadcast + reciprocal; the value matvec uses
    unnormalized exp weights and the 1/Z scale lands on the [1,512] psum
    result and on the transposed attn rows
"""
import sys

sys.path.insert(0, "/opt/trn_rl_repo")

import numpy as np
import ml_dtypes

import concourse.mybir as mybir
import concourse.tile as tile
from concourse import bacc
from concourse import bass_utils

BF16 = ml_dtypes.bfloat16
F32 = mybir.dt.float32
BF = mybir.dt.bfloat16

B, L, H = 16, 2048, 1024
NCORES = 8
BPC = B // NCORES          # batches per core = 2
T = BPC * L                # tokens per core = 4096
P = 128
HC = (2 * H) // P          # 16 contraction chunks (key 0..7, query 8..15)
OC = H // 512              # 2 free-dim halves of the output projection
NT = L // P                # 16 token tiles per batch
CHUNK = 256                # token chunk for transposed loads
NJ = CHUNK // P            # 4 token tiles per chunk


def build_nc():
    nc = bacc.Bacc(None, target_bir_lowering=False, debug=False)

    k_h = nc.dram_tensor("k", [T, H], BF, kind="ExternalInput")
    q_h = nc.dram_tensor("q", [T, H], BF, kind="ExternalInput")
    v_h = nc.dram_tensor("v", [T, H], BF, kind="ExternalInput")
    wcat_h = nc.dram_tensor("wcat", [2 * H, H], BF, kind="ExternalInput")
    biasw_h = nc.dram_tensor("biasw", [P, H], F32, kind="ExternalInput")
    wsc_h = nc.dram_tensor("wsc", [P, H], F32, kind="ExternalInput")
    nws_h = nc.dram_tensor("nws", [P, 1], F32, kind="ExternalInput")
    ident_h = nc.dram_tensor("ident", [P, P], F32, kind="ExternalInput")
    octx_h = nc.dram_tensor("octx", [BPC, H], F32, kind="ExternalOutput")
    oattn_h = nc.dram_tensor("oattn", [BPC, L], F32, kind="ExternalOutput")

    # per-batch chunk plan (token offsets within the batch): a small first
    # chunk on batch 0 so the first transposes land quickly
    def chunk_plan(b):
        if b == 0:
            return [(0, 256), (256, 256), (512, 512), (1024, 512), (1536, 512)]
        return [(off, 512) for off in range(0, L, 512)]

    with tile.TileContext(nc) as tc:
        with (
            tc.tile_pool(name="const", bufs=1) as const,
            tc.tile_pool(name="xt", bufs=3) as xtp,
            tc.tile_pool(name="vp", bufs=2) as vp,
            tc.tile_pool(name="work", bufs=4) as work,
            tc.tile_pool(name="up", bufs=2) as up,
            tc.tile_pool(name="small", bufs=4) as small,
            tc.tile_pool(name="psproj", bufs=4, space="PSUM") as psproj,
            tc.tile_pool(name="psctx", bufs=2, space="PSUM") as psctx,
            tc.tile_pool(name="pstiny", bufs=2, space="PSUM") as pstiny,
        ):
            # ---- HAM warm-up: dummy matmuls so the PE clock is ramping
            # toward 2.4 GHz while the first operands stream in.
            ones = const.tile([P, P], F32, tag="ones")
            nc.vector.memset(ones[:], 1.0)
            warm_ps = pstiny.tile([P, 1], F32, tag="tiny")
            for _ in range(32):
                nc.tensor.matmul(
                    warm_ps[:], ones[:], ones[:, 0:1], start=True, stop=True
                )

            # ---- transposed k/q loads, prefetchable ----
            xt_tiles = {}

            def issue_xt(b, ci):
                key = (b, ci)
                if key not in xt_tiles:
                    off, size = chunk_plan(b)[ci]
                    tt = b * L + off
                    xtk = xtp.tile([P, H // P, size], BF, tag="xtk")
                    nc.sync.dma_start_transpose(xtk[:], k_h[tt : tt + size, :])
                    xtq = xtp.tile([P, H // P, size], BF, tag="xtq")
                    nc.sync.dma_start_transpose(xtq[:], q_h[tt : tt + size, :])
                    xt_tiles[key] = (xtk, xtq)
                return xt_tiles[key]

            def issue_xt_flat(fi):
                n0 = len(chunk_plan(0))
                if fi < n0:
                    return issue_xt(0, fi)
                fi -= n0
                if fi < len(chunk_plan(1)):
                    return issue_xt(1, fi)
                return None

            # first chunk's transposes go out before everything else; weights
            # follow as per-chunk slices so MM hc=0 isn't gated on the full
            # 4MB weight load.
            issue_xt(0, 0)
            wcat = const.tile([P, HC, H], BF, tag="wcat")
            for hc in range(HC):
                nc.sync.dma_start(wcat[:, hc, :], wcat_h[hc * P : (hc + 1) * P, :])
            biasw = const.tile([P, H], F32, tag="biasw")
            nc.sync.dma_start(biasw[:], biasw_h[:, :])
            wsc = const.tile([P, H], F32, tag="wsc")
            nc.sync.dma_start(wsc[:], wsc_h[:, :])
            nws = const.tile([P, 1], F32, tag="nws")
            nc.sync.dma_start(nws[:], nws_h[:, :])
            ident = const.tile([P, P], F32, tag="ident")
            nc.sync.dma_start(ident[:], ident_h[:, :])

            flat_base = 0
            for b in range(BPC):
                t0 = b * L
                plan = chunk_plan(b)
                # value for this batch, natural layout [p, tile, h];
                # loaded chunk-by-chunk below so it never starves the
                # transpose stream
                v_sb = vp.tile([P, NT, H], BF, tag="v")

                score = up.tile([P, NT], F32, tag="score")
                sparts = up.tile([P, 2 * NT], F32, tag="sparts")
                u_f = up.tile([P, NT], F32, tag="uf")
                u_bf = up.tile([P, NT], BF, tag="ubf")
                usump = up.tile([P, len(plan)], F32, tag="usump")

                for ci, (off, size) in enumerate(plan):
                    xtk, xtq = issue_xt(b, ci)
                    for ahead in (1, 2):
                        issue_xt_flat(flat_base + ci + ahead)
                    nc.sync.dma_start(
                        v_sb[:, off // P : (off + size) // P, :],
                        v_h[t0 + off : t0 + off + size, :].rearrange(
                            "(c p) h -> p c h", p=P
                        ),
                    )

                    nj = size // P
                    c0 = off // P
                    for j in range(nj):
                        col = c0 + j
                        js = slice(j * P, (j + 1) * P)
                        for half in range(OC):
                            hs = slice(half * 512, (half + 1) * 512)
                            ps = psproj.tile([P, 512], F32, tag="proj")
                            for hc in range(HC):
                                src = xtk if hc < 8 else xtq
                                nc.tensor.matmul(
                                    ps[:],
                                    src[:, hc % 8, js],
                                    wcat[:, hc, hs],
                                    start=(hc == 0),
                                    stop=(hc == HC - 1),
                                )
                            biased = work.tile([P, 512], F32, tag="biased")
                            nc.vector.tensor_add(biased[:], ps[:], biasw[:, hs])
                            # tanh(x) = 2*sigmoid(2x) - 1 (ACT Tanh faults on
                            # this terminal; Sigmoid is accurate). score_true =
                            # 2*sum(w*sig) - sum(w); the *2 and -sum(w) fold
                            # into the Exp's scale/bias.
                            th = work.tile([P, 512], F32, tag="tanh")
                            nc.scalar.activation(
                                th[:],
                                biased[:],
                                mybir.ActivationFunctionType.Sigmoid,
                                scale=2.0,
                            )
                            nc.vector.tensor_mul(th[:], th[:], wsc[:, hs])
                            nc.vector.reduce_sum(
                                out=sparts[:, 2 * col + half : 2 * col + half + 1],
                                in_=th[:],
                                axis=mybir.AxisListType.X,
                            )
                    # incremental softmax for this chunk's columns:
                    # u = exp(2*s + nws), per-chunk partial sums in usump
                    nc.vector.reduce_sum(
                        out=score[:, c0 : c0 + nj].rearrange(
                            "p (c t) -> p c t", t=1
                        ),
                        in_=sparts[:, 2 * c0 : 2 * (c0 + nj)].rearrange(
                            "p (c t) -> p c t", t=2
                        ),
                        axis=mybir.AxisListType.X,
                    )
                    nc.scalar.activation(
                        u_f[:, c0 : c0 + nj],
                        score[:, c0 : c0 + nj],
                        mybir.ActivationFunctionType.Exp,
                        bias=nws[:, 0:1],
                        scale=2.0,
                        accum_out=usump[:, ci : ci + 1],
                    )
                    nc.vector.tensor_copy(
                        u_bf[:, c0 : c0 + nj], u_f[:, c0 : c0 + nj]
                    )
                flat_base += len(plan)

                # ---- batch tail: Z, 1/Z, outputs ----
                usum = up.tile([P, 1], F32, tag="usum")
                nc.vector.reduce_sum(
                    out=usum[:], in_=usump[:], axis=mybir.AxisListType.X
                )
                zb = pstiny.tile([P, 1], F32, tag="tiny")
                nc.tensor.matmul(zb[:], ones[:], usum[:], start=True, stop=True)
                rz = small.tile([P, 1], F32, tag="rz")
                nc.vector.reciprocal(rz[:], zb[:])

                # ---- context = (1/Z) * sum_t u[t] * v[t, :] ----
                for half in range(OC):
                    hs = slice(half * 512, (half + 1) * 512)
                    cps = psctx.tile([1, 512], F32, tag="ctx")
                    for t in range(NT):
                        nc.tensor.matmul(
                            cps[:],
                            u_bf[:, t : t + 1],
                            v_sb[:, t, hs],
                            start=(t == 0),
                            stop=(t == NT - 1),
                        )
                    ctx_sb = small.tile([1, 512], F32, tag="ctxsb")
                    nc.vector.tensor_scalar_mul(ctx_sb[:], cps[:], rz[0:1, 0:1])
                    nc.sync.dma_start(octx_h[b, hs], ctx_sb[:])

                # ---- attn output: transpose u_f -> [16, 128] rows, then
                # scale by 1/Z per output row (rz is partition-replicated) ----
                at = pstiny.tile([NT, P], F32, tag="tiny")
                nc.tensor.transpose(at[:], u_f[:], ident[:])
                attn_sb = small.tile([NT, P], F32, tag="attnsb")
                nc.vector.tensor_scalar_mul(attn_sb[:], at[:], rz[0:NT, 0:1])
                nc.sync.dma_start(
                    oattn_h[b, :].rearrange("(c p) -> c p", c=NT), attn_sb[:]
                )

    nc.compile()
    return nc


_NC_CACHE = None


def _get_nc():
    global _NC_CACHE
    if _NC_CACHE is None:
        _NC_CACHE = build_nc()
    return _NC_CACHE


def _prep_inputs(query, key_t, value, Wq, Wk, bias, w_score):
    query = np.asarray(query, dtype=np.float32)
    key_t = np.asarray(key_t, dtype=np.float32)
    value = np.asarray(value, dtype=np.float32)
    k_b = key_t.reshape(NCORES, T, H).astype(BF16)
    q_b = query.reshape(NCORES, T, H).astype(BF16)
    v_b = value.reshape(NCORES, T, H).astype(BF16)
    wcat = np.ascontiguousarray(
        np.concatenate(
            [np.asarray(Wk, np.float32).T, np.asarray(Wq, np.float32).T], axis=0
        )
    ).astype(BF16)
    biasw = np.ascontiguousarray(
        np.tile(np.asarray(bias, np.float32)[None, :], (P, 1))
    )
    wsc = np.ascontiguousarray(
        np.tile(np.asarray(w_score, np.float32)[None, :], (P, 1))
    )
    nws = np.full((P, 1), -np.asarray(w_score, np.float64).sum(), dtype=np.float32)
    ident = np.eye(P, dtype=np.float32)
    return [
        {
            "k": np.ascontiguousarray(k_b[i]),
            "q": np.ascontiguousarray(q_b[i]),
            "v": np.ascontiguousarray(v_b[i]),
            "wcat": wcat,
            "biasw": biasw,
            "wsc": wsc,
            "nws": nws,
            "ident": ident,
        }
        for i in range(NCORES)
    ]


def kernel(query, key_t, value, Wq, Wk, bias, w_score, b_score, _trace=False):
    in_maps = _prep_inputs(query, key_t, value, Wq, Wk, bias, w_score)
    nc = _get_nc()
    res = bass_utils.run_bass_kernel_spmd(
        nc, in_maps, core_ids=list(range(NCORES)), trace=_trace
    )
    ctx = np.stack([r["octx"] for r in res.results]).reshape(B, H)[:, None, :]
    attn = np.stack([r["oattn"] for r in res.results]).reshape(B, L)
    if _trace:
        kernel.last_results = res
    return ctx, attn


if __name__ == "__main__":
    rng = np.random.default_rng(0)
    ins = {
        "query": rng.standard_normal((B, L, H), dtype=np.float32),
        "key_t": rng.standard_normal((B, L, H), dtype=np.float32),
        "value": rng.standard_normal((B, L, H), dtype=np.float32),
        "Wq": rng.uniform(-1 / 32, 1 / 32, (H, H)).astype(np.float32),
        "Wk": rng.uniform(-1 / 32, 1 / 32, (H, H)).astype(np.float32),
        "bias": rng.uniform(-0.1, 0.1, (H,)).astype(np.float32),
        "w_score": rng.uniform(-1 / 32, 1 / 32, (H,)).astype(np.float32),
        "b_score": rng.uniform(-1 / 32, 1 / 32, (1,)).astype(np.float32),
    }
    ctx, attn = kernel(**ins)
    print("ctx", ctx.shape, "attn", attn.shape, attn.sum(-1)[:4])
